# revision 21
# baseline (speedup 1.0000x reference)
"""Dual (real/imag magnitude) attention on 8 TRN2 NeuronCores.

Problem: B=2, H=16, S=2048, D=64 (per b,h):
  scores = sqrt((Q K^T)^2 + (Qi Ki^T)^2 + 1e-8) / 8
  p = softmax(where(mask==0, -1e9, scores));  out = (p V, p Vi)

Strategy: data-parallel over the 32 (b,h) pairs -> 4 pairs/core, no
collectives.  Scores are computed TRANSPOSED ([k, q] layout) so the
softmax matrix feeds matmul-2 directly as the moving operand.

v2 pipeline (one elementwise pass per engine class):
  PE   : ps_r = K^T Q (rows 0-63), ps_i = Ki^T Qi (rows 64-127),
         row-packed and concurrent; later MM2 + denominator matmuls.
  DVE  : ONE custom op SQSQ: u = ps_r^2 + ps_i^2 (two PSUM sources,
         fp16 out).  No scaling - the 1/64 is folded into the table.
  DVE/GPS: mask penalty u += -60000 (tensor_tensor add, split by kc;
         DVE gets 2x_1p fp16 perf mode, GPS takes the larger share).
  ACT  : p = exp(sqrt(u)/8) in ONE pass over [128, 4096] chunks via a
         patched PWP table (sqrt slot rebuilt with cubic Taylor coeffs
         of exp(sqrt(x)/8); negative (masked) inputs -> 0.0, tiny
         positive -> 1.0).  Masked p == 0 exactly.
  PE   : out[dd, q] += [V|Vi]^T[kc] @ P[kc]; dnm via ones-weight matmul.
  GPS  : PSUM -> SBUF copies of MM2 outputs; host divides by dnm.
"""

import os
import sys
import types

import numpy as np

B, H, S, D = 2, 16, 2048, 64
N_CORES = 8
PAIRS = 4           # (b,h) pairs per core
KC = S // 128       # 16 k-chunks of 128
HALF = S // 2       # q processed in halves of 1024
# kcs whose r-square (op1) runs on ScalarE (ACT); their mask penalty is
# added by GpSimd post-op2.  The rest run op1 on DVE as SQPLUS with the
# penalty folded into in1 for free.  kcs 0-4 (incl. ALL of exp chunk 0)
# stay on DVE so the first exp chunk has no GpSimd dependency and the
# strict-FIFO ACT queue never head-blocks early in a half.
ACT_SQ_KC = frozenset(range(5, 16))


def _ensure_axon_hooks():
    try:
        import antenv.axon_hooks  # noqa: F401
        return
    except ImportError:
        pass
    mod = types.ModuleType("antenv.axon_hooks")

    def set_axon_ntff_profile_hook(h):
        mod._hook = h

    def get_axon_ntff_profile_hook():
        return getattr(mod, "_hook", None)

    mod.set_axon_ntff_profile_hook = set_axon_ntff_profile_hook
    mod.get_axon_ntff_profile_hook = get_axon_ntff_profile_hook
    sys.modules["antenv.axon_hooks"] = mod
    try:
        import antenv
        antenv.axon_hooks = mod
        from trn_agent_boot.trn_boot import _ntff_profile_via_ctypes
        set_axon_ntff_profile_hook(_ntff_profile_via_ctypes("/opt/axon/libaxon_pjrt.so"))
    except Exception:
        pass


def _register_sqplus():
    """SQPLUS: out = in0*in0 + in1 (one PSUM source + one SBUF source —
    the DVE may read at most ONE non-scalar input from PSUM)."""
    import concourse.dve_ops as dvo
    from concourse.dve_spec import Bin, AluOp, Spec, Src0, Src1, sq, _has_src1, lower
    from concourse.dve_uop import DveOpSpec

    name = "SQPLUS1_ANT"
    if name in dvo._SUB_OPCODE_FOR_NAME:
        return next(op for op in dvo.OPS if op.name == name)
    spec = Spec(
        body=Bin(AluOp.ADD, sq(Src0), Src1),
        reference=lambda in0, in1, s0, s1, imm2: in0 * in0 + in1,
    )
    opcode = dvo._CUSTOM_DVE_ROW_BASE + len(dvo.OPS)
    shas = {}
    for ver in ("v3", "v4"):
        s = DveOpSpec(name=name, opcode=opcode, uops=lower(spec, ver=ver),
                      rd1_en=_has_src1(spec))
        shas[ver] = s.sha(ver)
    op = dvo.DveOp(name, spec, subdim=False, uops_sha=shas)
    dvo.OPS.append(op)
    dvo.CUSTOM_DVE_SPECS[name] = spec
    dvo._SUB_OPCODE_FOR_NAME[name] = opcode
    return op


_BUILT = None


_PWP_DST = "/tmp/mypwp_expsqrt_v3"


def _patch_pwp_tables():
    """Build an act-table root where the `sqrt` function's PWP buckets
    compute exp(sqrt(x)/8) instead (same centers, new Taylor coeffs).
    activation(func=Sqrt) then maps raw u = r^2 + i^2 directly to the
    softmax numerator exp(score) in ONE ScalarE pass:
      - exps -48..13: cubic Taylor of f(x) = exp(sqrt(x)/8) at the
        original bucket centers (legit u <= ~4050 < 2^12).
      - exps < -48: constant 1.0 (f(0); avoids f32 coeff overflow from
        the x^-2.5 term of the cubic at tiny centers).
      - exps > 13: constant 60000 (unreachable; fp16-safe cap).
      - negative inputs (mask penalty -60000 added to u): exactly 0.0.
      - tiny-positive special region: 1.0 = f(0)."""
    import json
    import shutil

    import neuronxcc

    src_dir = os.path.join(os.path.dirname(neuronxcc.__file__),
                           "pwp", "pwp_bin_trainium")
    if not os.path.exists(os.path.join(_PWP_DST, "act_info.json")):
        tmp = _PWP_DST + ".tmp%d" % os.getpid()
        if os.path.exists(tmp):
            shutil.rmtree(tmp)
        shutil.copytree(src_dir, tmp)
        os.chmod(tmp, 0o755)
        for f in os.listdir(tmp):
            os.chmod(os.path.join(tmp, f), 0o644)
        d = tmp + "/"
        j = json.load(open(d + "sqrt_and_others.json"))
        raw = open(d + "sqrt_and_others_bkt.bin", "rb").read()
        arr = np.frombuffer(raw, dtype=np.float32).reshape(-1, 8).copy()
        m = j["func_exp_to_bkt_start_idx"]["sqrt"]
        idxs = sorted((int(es), v[0]) for es, v in m.items())
        idxs.append((10**9, len(arr)))  # sentinel end
        for (e, start), (e2, start2) in zip(idxs, idxs[1:]):
            if e >= 10**9:
                break
            for i in range(start, start2):
                if e < -48:
                    arr[i][0:4] = [1.0, 0.0, 0.0, 0.0]
                elif e > 13:
                    arr[i][0:4] = [60000.0, 0.0, 0.0, 0.0]
                else:
                    c = float(arr[i][4])
                    if c <= 0.0:
                        arr[i][0:4] = [1.0, 0.0, 0.0, 0.0]
                        continue
                    t = np.sqrt(c)
                    F = np.exp(t / 8.0)
                    arr[i][0] = F
                    arr[i][1] = F / (16.0 * t)
                    arr[i][2] = F * (t - 8.0) / (512.0 * t**3)
                    arr[i][3] = F * (t * t - 24.0 * t + 192.0) / (24576.0 * t**5)
        # special-region buckets from profile_meta_data
        prof = next(p for p in j["profile_meta_data"]
                    if p["func_name"].startswith("sqrt"))
        for key, val in (("pos_small_signal_pwl_control", 1.0),
                         ("pos_large_signal_pwl_control", 60000.0),
                         ("neg_small_signal_pwl_control", 0.0),
                         ("neg_large_signal_pwl_control", 0.0)):
            bi = prof[key]
            arr[bi][0] = val
            arr[bi][1] = 0.0
            arr[bi][2] = 0.0
            arr[bi][3] = 0.0
            arr[bi][4] = 0.0
        open(d + "sqrt_and_others_bkt.bin", "wb").write(arr.tobytes())
        try:
            os.rename(tmp, _PWP_DST)
        except OSError:
            shutil.rmtree(tmp)  # another process won the race

    os.environ["BASS_ACT_ROOT_JSON_PATH"] = os.path.join(
        _PWP_DST, "act_info.json")
    # the act-root override is not part of the NEFF cache key
    os.environ["NEURON_FORCE_RECOMPILE"] = "1"


def _build():
    global _BUILT
    if _BUILT is not None:
        return _BUILT
    _ensure_axon_hooks()
    _patch_pwp_tables()
    SQPLUS = _register_sqplus()

    from concourse import bacc, mybir, tile

    f16 = mybir.dt.float16
    f32 = mybir.dt.float32
    AF = mybir.ActivationFunctionType
    ADD = mybir.AluOpType.add

    nc = bacc.Bacc("TRN2", target_bir_lowering=False, debug=False,
                   num_devices=N_CORES)
    qt_ext = nc.declare_dram_parameter("qt", [PAIRS, 128, S], f16, isOutput=False)
    kt_ext = nc.declare_dram_parameter("kt", [PAIRS, 128, S], f16, isOutput=False)
    vv_ext = nc.declare_dram_parameter("vv", [PAIRS, 128, KC, 128], f16,
                                       isOutput=False)
    pen_ext = nc.declare_dram_parameter("pen", [128, KC, S], f16, isOutput=False)
    out_ext = nc.declare_dram_parameter("out", [PAIRS, 2, 128, HALF], f32,
                                        isOutput=True)
    dnm_ext = nc.declare_dram_parameter("dnm", [PAIRS, 2, HALF], f32,
                                        isOutput=True)

    with tile.TileContext(nc) as tc:
        with (
            tc.tile_pool(name="resident", bufs=1) as resident,
            tc.tile_pool(name="upool", bufs=2) as upool,
            tc.tile_pool(name="sqp", bufs=2) as sqp,
            tc.tile_pool(name="oc", bufs=2) as oc,
            tc.tile_pool(name="psr", bufs=2, space="PSUM") as psr,
            tc.tile_pool(name="psi", bufs=2, space="PSUM") as psi,
            tc.tile_pool(name="ps2", bufs=1, space="PSUM") as ps2,
            tc.tile_pool(name="psd", bufs=1, space="PSUM") as psd,
        ):
            # -------- resident inputs: all 4 pairs up front ----------
            qt_t = resident.tile([128, PAIRS, S], f16)
            kt_t = resident.tile([128, PAIRS, S], f16)
            vv_t = resident.tile([128, PAIRS, KC, 128], f16)
            pen_t = resident.tile([128, KC, S], f16)
            ones_t = resident.tile([128, 1], f16)

            # pair-0 q/k first (first matmuls wait on them), then pen in
            # kc order (pen(kc) is needed shortly after its MM1s), then
            # the rest.
            nc.sync.dma_start(qt_t[:, 0, :], qt_ext[0])
            nc.sync.dma_start(kt_t[:, 0, :], kt_ext[0])
            nc.gpsimd.memset(ones_t[:], 1.0)
            for kc in range(KC):
                nc.sync.dma_start(pen_t[:, kc, :], pen_ext[:, kc, :])
            nc.sync.dma_start(vv_t[:, 0, :, :], vv_ext[0])
            for p in range(1, PAIRS):
                nc.sync.dma_start(qt_t[:, p, :], qt_ext[p])
                nc.sync.dma_start(kt_t[:, p, :], kt_ext[p])
                nc.sync.dma_start(vv_t[:, p, :, :], vv_ext[p])

            # -------- per-half work ----------------------------------
            # Slot queue: half i owns slots [i*32, (i+1)*32); its MM2/dn
            # trail into half i+1's slots.  Tile derives deps from
            # emission order, so producers always precede consumers.
            work = {}

            def add(s, fn):
                work.setdefault(s, []).append(fn)

            halves = [(p, h) for p in range(PAIRS) for h in range(2)]
            for i, (p, h) in enumerate(halves):
                base = i * 32
                u_t = upool.tile([128, KC, HALF], f16, name="u_t")
                o_t = oc.tile([128, HALF], f32, name="o_t")
                d_t = oc.tile([1, HALF], f32, name="d_t")
                hs = h * HALF

                # ---- phase A: MM1 + squares + pen ----
                # op1 (per qn): sq = ps_r^2 (+pen) -> sq2[:, qn, :] fp16
                #   DVE kcs: SQPLUS(ps_r, pen) folds the penalty for free
                #   ACT kcs: ScalarE Square; penalty added by GpSimd later
                # op2 (per kc, wide): u[:, kc, :] = ps_i2^2 + sq2 over both
                #   qn at once ([128, 2, 512] two-bank PSUM AP).
                sq_state = {}
                for kc in range(KC):
                    for qn in range(2):
                        def a_step(kc=kc, qn=qn, u_t=u_t, p=p, hs=hs,
                                   sq_state=sq_state):
                            qs = hs + qn * 512
                            ksl = slice(kc * 128, (kc + 1) * 128)
                            if qn == 0:
                                sq_state[0] = psi.tile([128, 2, 512], f32,
                                                       name="ps_i2")
                                sq_state[1] = sqp.tile([128, 2, 512], f16,
                                                       name="sq2")
                            ps_i2, sq2 = sq_state[0], sq_state[1]
                            ps_r = psr.tile([128, 512], f32)
                            nc.tensor.matmul(ps_r[:], kt_t[0:64, p, ksl],
                                             qt_t[0:64, p, qs:qs + 512],
                                             start=True, stop=True,
                                             tile_position=(0, 0))
                            nc.tensor.matmul(ps_i2[:, qn, :],
                                             kt_t[64:128, p, ksl],
                                             qt_t[64:128, p, qs:qs + 512],
                                             start=True, stop=True,
                                             tile_position=(64, 0))
                            if kc in ACT_SQ_KC:
                                nc.scalar.square(sq2[:, qn, :], ps_r[:])
                            else:
                                nc.vector._custom_dve(
                                    SQPLUS, out=sq2[:, qn, :], in0=ps_r[:],
                                    in1=pen_t[:, kc, qs:qs + 512])
                            if qn == 1:
                                nc.vector._custom_dve(
                                    SQPLUS, out=u_t[:, kc, :],
                                    in0=ps_i2[:, :, :], in1=sq2[:, :, :])
                        add(base + 2 * kc + qn, a_step)

                    if kc in ACT_SQ_KC:
                        def pen_step(kc=kc, u_t=u_t, hs=hs):
                            nc.gpsimd.tensor_tensor(
                                u_t[:, kc, :], u_t[:, kc, :],
                                pen_t[:, kc, hs:hs + HALF], ADD)
                        add(base + 2 * kc + 1, pen_step)

                # ---- phase B: exp chunks (4 kc each) ----
                # chunk 3 is emitted just past the half boundary so its
                # GpSimd pen wait doesn't block the next half's squares.
                for c in range(4):
                    def b_step(c=c, u_t=u_t):
                        nc.scalar.activation(u_t[:, 4 * c:4 * c + 4, :],
                                             u_t[:, 4 * c:4 * c + 4, :],
                                             AF.Sqrt, bias=0.0, scale=1.0)
                    add(base + (8 * c + 8 if c < 3 else 33), b_step)

                # ---- phase C: MM2 + dn (into next half's slots) ----
                # Each qn's MM2 chain is ONE contiguous 16-matmul burst
                # (~7us of back-to-back PE work) so the HAM activity
                # window sees sustained busy and un-gates the PE clock to
                # 2.4 GHz.  The softmax denominator: after a qn's burst
                # consumed its p values, an IN-PLACE pairwise kc-add tree
                # (DVE 2x fp16 wide stages, GpSimd short stages) reduces
                # u_t[:, 0:16, qsl] to u_t[:, 0, qsl]; one tiny matmul
                # then computes dn = ones^T @ presum.
                cstate = {}

                def mm2_burst(qn, u_t=u_t, p=p, cstate=cstate):
                    qsl = slice(qn * 512, (qn + 1) * 512)
                    if qn == 0:
                        cstate["dn"] = psd.tile([128, 512], f32, name="dn")
                    cstate["po"] = ps2.tile([128, 512], f32, name="po")
                    po = cstate["po"]
                    for kc in range(KC):
                        nc.tensor.matmul(po[:], vv_t[:, p, kc, :],
                                         u_t[:, kc, qsl],
                                         start=(kc == 0),
                                         stop=(kc == KC - 1))

                def o_copy(qn, o_t=o_t, cstate=cstate):
                    qsl = slice(qn * 512, (qn + 1) * 512)
                    nc.scalar.copy(o_t[:, qsl], cstate["po"][:])

                def ps_step(st, qn, u_t=u_t):
                    qsl = slice(qn * 512, (qn + 1) * 512)
                    w = 8 >> st
                    eng = nc.vector if st < 2 else nc.gpsimd
                    eng.tensor_tensor(u_t[:, 0:w, qsl], u_t[:, 0:w, qsl],
                                      u_t[:, w:2 * w, qsl], ADD)

                def dn_step(qn, u_t=u_t, d_t=d_t, cstate=cstate):
                    qsl = slice(qn * 512, (qn + 1) * 512)
                    dsl = slice(32 * qn, 32 * qn + 1)
                    nc.tensor.matmul(cstate["dn"][dsl, :], ones_t[:],
                                     u_t[:, 0, qsl], start=True, stop=True)
                    nc.scalar.copy(d_t[:, qsl], cstate["dn"][dsl, :])

                nb = base + 32
                add(nb + 6, (lambda f=mm2_burst: f(0)))
                add(nb + 13, (lambda f=o_copy: f(0)))
                add(nb + 14, (lambda f=mm2_burst: f(1)))
                for st in range(4):
                    add(nb + 16 + st, (lambda st=st, f=ps_step: f(st, 0)))
                add(nb + 20, (lambda f=dn_step: f(0)))
                add(nb + 22, (lambda f=o_copy: f(1)))
                for st in range(4):
                    add(nb + 24 + st, (lambda st=st, f=ps_step: f(st, 1)))
                add(nb + 28, (lambda f=dn_step: f(1)))

                def out_step(p=p, h=h, o_t=o_t, d_t=d_t):
                    nc.sync.dma_start(out_ext[p, h], o_t[:])
                    nc.sync.dma_start(dnm_ext[p, h], d_t[:])
                add(nb + 30, out_step)

            for s in sorted(work):
                for fn in work[s]:
                    fn()

    nc.compile()
    _BUILT = nc
    return nc


LAST_EXEC_NS = None


def kernel(query, key, value, query_i, key_i, value_i, mask):
    global LAST_EXEC_NS
    nc = _build()
    from concourse.bass_utils import run_bass_kernel_spmd

    q = np.asarray(query, dtype=np.float32)
    k = np.asarray(key, dtype=np.float32)
    v = np.asarray(value, dtype=np.float32)
    qi = np.asarray(query_i, dtype=np.float32)
    ki = np.asarray(key_i, dtype=np.float32)
    vi = np.asarray(value_i, dtype=np.float32)
    m = np.asarray(mask)

    in_maps = []
    for c in range(N_CORES):
        b = (c * PAIRS) // H
        h0 = (c * PAIRS) % H
        qt = np.empty((PAIRS, 128, S), np.float16)
        kt = np.empty((PAIRS, 128, S), np.float16)
        vv = np.empty((PAIRS, 128, KC, 128), np.float16)
        for p in range(PAIRS):
            hh = h0 + p
            qt[p, 0:64] = q[b, hh].T
            qt[p, 64:128] = qi[b, hh].T
            kt[p, 0:64] = k[b, hh].T
            kt[p, 64:128] = ki[b, hh].T
            vvp = np.concatenate([v[b, hh], vi[b, hh]], axis=1)  # [S, 128]
            # [S, 128] -> [128 part, KC, 128 dd] with S = KC*128
            vv[p] = vvp.reshape(KC, 128, 128).transpose(1, 0, 2)
        # additive mask penalty: masked scores get -6e4 (still finite in
        # fp16) so the patched exp(sqrt(x)/8) table maps them to exactly 0
        pen = np.where(m[b, 0].T == 0, np.float16(-60000.0), np.float16(0.0))
        pen = pen.reshape(KC, 128, S).transpose(1, 0, 2).copy()
        in_maps.append({"qt": qt, "kt": kt, "vv": vv, "pen": pen})

    res = run_bass_kernel_spmd(nc, in_maps, list(range(N_CORES)))
    LAST_EXEC_NS = res.exec_time_ns

    real = np.empty((B, H, S, D), np.float32)
    img = np.empty((B, H, S, D), np.float32)
    for c in range(N_CORES):
        b = (c * PAIRS) // H
        h0 = (c * PAIRS) % H
        o = res.results[c]["out"]     # [PAIRS, 2, 128, HALF]
        dn = res.results[c]["dnm"]    # [PAIRS, 2, HALF]
        for p in range(PAIRS):
            od = o[p] / dn[p][:, None, :]          # [2, 128, HALF]
            full = np.concatenate([od[0], od[1]], axis=1)  # [128, S]
            real[b, h0 + p] = full[0:64].T
            img[b, h0 + p] = full[64:128].T
    return (real, img)
